# revision 2
# baseline (speedup 1.0000x reference)
"""Complex-valued attention kernel for Trainium2, SPMD over 8 NeuronCores.

Problem (hardcoded shapes): B=4, N=2048, E=384, H=6, D=64, complex64.
  qkv = x @ w_qkv^T + b_qkv          (complex)
  q, k = complex RMSNorm over D (eps=1e-6), affine weights qn_w/kn_w
  scores = Re(q @ conj(k)^T) / sqrt(D)
  attn = softmax(scores)  (real), out = attn @ v   -> [B, N, E] complex64

Sharding: core c handles batch b=c//2, heads 3*(c%2)..3*(c%2)+2 (24 head-
batches over 8 cores, 3 each).

Design notes (HW ~201us vs 365us baseline; trace-verified on trn2):
  - PH1 (QKV+norm+pack, ~63us): bias folded into the PSUM->SBUF evacuation
    adds (DVE tensor_tensor, f32 bias broadcast on-device via gpsimd);
    q scaled by rq on ScalarE copies; k left UNSCALED -- rk is folded into
    the PH2 exp as a per-partition activation scale (softmax row scale is
    per-kv-token there, which is the partition dim of S^T).  Token->pack
    transposes via the DMA XBAR (dma_start_transpose, contiguous
    [128,3,128] dest -- a strided dest produces wrong data on HW) instead
    of PE transpose + DVE copy.  V bias applied on HOST (sum(attn)=1 makes
    it a constant output offset).
  - PH2 (attention, ~115us): per (head, 1024-q chunk) x 16 kv tiles:
    S^T = kpack.T @ qpack (one 128-contraction MM per 512 free), exp with
    scale=rk on ScalarE, PV accumulation.  Z row sums NOT on the PE:
    exp tiles are accumulated on DVE into two bf16 zacc buffers, DMA'd
    out raw; the 128-partition Z sum, 1/Z division and out^T -> [token,d]
    transpose all happen on host during unsharding.  PSUM budget:
    st(2 bufs x 2 banks) + pv(2 bufs x 2 banks) = 8 banks -> consecutive
    chunks overlap and the PE never idles long enough for HAM re-throttle
    (K=8/8 at 2.4 GHz through the whole phase).
  - ScalarE exp is the hard floor: 96 x [128,1024] x ~1.2us ~= 115us.
    PSUM (8 banks) rules out wider exp granules; DVE/GpSimd cannot exp.
  - Known walrus/HW landmines (do NOT reintroduce): tensor_tensor_reduce
    (HW exec fault), gpsimd tensor ops reading PSUM (walrus crash),
    rearrange-view reduce over [128,(h c)] (NaN on HW, fine in CoreSim),
    gpsimd tensor_scalar_mul (runs but ~2.1us per [128,128] -- 6x slower
    than DVE/ACT; regressed the kernel to 281us).  Also measured neutral-
    to-negative: k-square on gpsimd + ph1 psum bufs 8->6 (201.6us vs 196).
    The PH1->PH2 transition warm-keeper dummies fix the HAM re-throttle
    there (trace-confirmed) but the wall is ACT-gated at that point, so
    the gain is variance reduction only.
"""

import numpy as np

import concourse.bass as bass
import concourse.tile as tile
from concourse import bacc, mybir
from concourse.bass_utils import run_bass_kernel_spmd

B, N, E, H, D = 4, 2048, 384, 6, 64
EPS = 1e-6
HPC = 3            # heads per core
NT = N // 128      # 16 token tiles
KT = E // 128      # 3 contraction tiles
QC = 2             # q chunks of 1024
F32 = mybir.dt.float32
MMD = mybir.dt.bfloat16

_prog_cache = {}

# Schraudolph exp on DVE for these kv tiles of every (head, q-chunk):
# i16 = round(S^T * (rk*A) + SCH_B), bitcast int16->bf16 gives exp to ~2%.
# Offloads ACT (the PH2 pacer) onto DVE slack.
SCH_A = float(2.0 ** 7 / np.log(2.0))
SCH_B = float(127 * 2 ** 7 - 7.4)
SCH_KT = (2, 6, 10, 14)


def _widx(p, a, k):
    return (p * 2 + a) * KT + k


def build_program():
    nc = bacc.Bacc(
        "TRN2", target_bir_lowering=False, debug=False, num_devices=8)
    xt_r = nc.declare_dram_parameter("xt_r", [E, N], MMD, isOutput=False)
    xt_i = nc.declare_dram_parameter("xt_i", [E, N], MMD, isOutput=False)
    w_in = nc.declare_dram_parameter("w", [3, 2, E, 384], MMD, isOutput=False)
    b_in = nc.declare_dram_parameter("bias", [1, 2, 384], F32, isOutput=False)
    outT_d = nc.declare_dram_parameter("outT", [HPC, QC, 128, 1024], F32,
                                       isOutput=True)
    zac_d = nc.declare_dram_parameter("zac", [HPC, QC, 128, 2, 1024], MMD,
                                      isOutput=True)

    with tile.TileContext(nc) as tc:
        with tc.tile_pool(name="persist", bufs=1) as pp:
            bias_row = pp.tile([1, 2, 384], F32)
            nc.sync.dma_start(out=bias_row, in_=b_in[:])
            bias_sb = pp.tile([128, 2, 384], F32)
            nc.gpsimd.partition_broadcast(bias_sb, bias_row)
            w_sb = pp.tile([128, 3 * 2 * KT, 384], MMD)

            # packs: [d2, token-tile, head, token-in-tile]
            qpack = pp.tile([128, NT, HPC, 128], MMD)
            kpack = pp.tile([128, NT, HPC, 128], MMD)
            vpack = pp.tile([128, NT, 384], MMD)   # [token, kv-tile, head*128]
            rk_sb = pp.tile([128, NT, HPC], F32)   # per-kv-token k norm scale
            rkA_sb = pp.tile([128, NT, HPC], F32)  # rk * SCH_A (DVE exp tiles)
            eps_q = pp.tile([128, 1], F32)
            eps_k = pp.tile([128, 1], F32)
            nc.vector.memset(eps_q, 64.0 * EPS)
            nc.vector.memset(eps_k, EPS)

            xt_sb = pp.tile([128, 2, KT, N], MMD)
            NXC = 8  # x dma chunks

            def _x_chunk(xc, split):
                # split=True: x_r via the ACT HWDGE queue, x_i via SP, so
                # the SP queue has room for the 32 pack transposes.
                sl = slice(xc * (N // NXC), (xc + 1) * (N // NXC))
                eng_r = nc.scalar if split else nc.sync
                eng_r.dma_start(
                    out=xt_sb[:, 0, :, sl],
                    in_=xt_r[:, sl].rearrange("(k q) n -> q k n", q=128),
                )
                nc.sync.dma_start(
                    out=xt_sb[:, 1, :, sl],
                    in_=xt_i[:, sl].rearrange("(k q) n -> q k n", q=128),
                )

            # x chunk 0 first (gates tile 0), then weights in 6 chunks
            # ordered by per-tile MM consumption ((k,a) inner loops) so the
            # first matmuls start ~4us in instead of waiting for all 3.5MB,
            # then the remaining x chunks.
            _x_chunk(0, split=False)
            for k in range(KT):
                for a in range(2):
                    nc.sync.dma_start(
                        out=w_sb[:, a * KT + k::2 * KT],
                        in_=w_in[:, a, k * 128:(k + 1) * 128, :].rearrange(
                            "p q c -> q p c"),
                    )
            for xc in range(1, NXC):
                _x_chunk(xc, split=True)

            # HAM pre-warm: dummy matmuls keep the PE busy through the
            # initial DMA wait so the 4096-cycle activity window fires and
            # PH1's real matmuls run at K=8/8 (2.4 GHz) from the start.
            wz = pp.tile([128, 512], MMD)
            nc.vector.memset(wz, 0.0)
            with tc.tile_pool(name="warm", bufs=1, space="PSUM") as pwm:
                warm_ps = pwm.tile([128, 512], F32)
                for _ in range(14):
                    nc.tensor.matmul(warm_ps, wz[:, 0:128], wz,
                                     start=True, stop=True)

            # ---------------- PH1: QKV + RMS norm + packing ----------------
            with (
                tc.tile_pool(name="ph1ps", bufs=8, space="PSUM") as pps,
                tc.tile_pool(name="ph1t", bufs=4) as pt1,
            ):
                for nt in range(NT):
                    psq = pps.tile([128, 384], F32, tag="ps")
                    psk = pps.tile([128, 384], F32, tag="ps")
                    psv = pps.tile([128, 384], F32, tag="ps")
                    for k in range(KT):
                        for a in range(2):
                            lhs = xt_sb[:, a, k, nt * 128:(nt + 1) * 128]
                            st = (k == 0 and a == 0)
                            sp = (k == KT - 1 and a == 1)
                            nc.tensor.matmul(psq, lhs, w_sb[:, _widx(0, a, k)],
                                             start=st, stop=sp)
                            nc.tensor.matmul(psk, lhs, w_sb[:, _widx(1, a, k)],
                                             start=st, stop=sp)
                            nc.tensor.matmul(psv, lhs, w_sb[:, _widx(2, a, k)],
                                             start=st, stop=sp)
                    # V: plain PSUM->SBUF bf16 evacuation (bias on host);
                    # on DVE to keep the ACT queue under the PE pace in PH1.
                    nc.vector.tensor_copy(vpack[:, nt], psv)
                    # Q/K: add bias during evacuation
                    q2 = pt1.tile([128, 384], F32, tag="q2")
                    k2s = pt1.tile([128, 384], MMD, tag="k2s")
                    nc.vector.tensor_add(q2, psq, bias_sb[:, 0])
                    nc.vector.tensor_add(k2s, psk, bias_sb[:, 1])
                    # sum of squares per head: squares on gpsimd, reduce DVE
                    scr = pt1.tile([128, 384], MMD, tag="scr")
                    scrk = pt1.tile([128, 384], MMD, tag="scrk")
                    msq = pt1.tile([128, HPC], F32, tag="msq")
                    msk = pt1.tile([128, HPC], F32, tag="msk")
                    nc.scalar.square(scr, q2)
                    nc.vector.tensor_mul(scrk, k2s, k2s)
                    for hh in range(HPC):
                        blk = slice(hh * 128, (hh + 1) * 128)
                        nc.vector.reduce_sum(msq[:, hh:hh + 1], scr[:, blk],
                                             axis=mybir.AxisListType.X)
                        nc.vector.reduce_sum(msk[:, hh:hh + 1], scrk[:, blk],
                                             axis=mybir.AxisListType.X)
                    # q: rq = 1/(8 sqrt(ms+eps)) = 1/sqrt(sum_sq + 64 eps)
                    # k: rk = 1/sqrt(ms+eps)     = 1/sqrt((sum_sq)/64 + eps)
                    s8q = pt1.tile([128, HPC], F32, tag="s8q")
                    s8k = pt1.tile([128, HPC], F32, tag="s8k")
                    nc.scalar.activation(s8q, msq,
                                         mybir.ActivationFunctionType.Sqrt,
                                         bias=eps_q, scale=1.0)
                    nc.scalar.activation(s8k, msk,
                                         mybir.ActivationFunctionType.Sqrt,
                                         bias=eps_k, scale=1.0 / 64.0)
                    rq = pt1.tile([128, HPC], F32, tag="rq")
                    nc.vector.reciprocal(rq, s8q)
                    nc.vector.reciprocal(rk_sb[:, nt], s8k)
                    nc.vector.tensor_scalar_mul(rkA_sb[:, nt], rk_sb[:, nt],
                                                SCH_A)
                    # scale q by rq (per-head per-token) while casting to bf16
                    q2s = pt1.tile([128, 384], MMD, tag="q2s")
                    for hh in range(HPC):
                        blk = slice(hh * 128, (hh + 1) * 128)
                        nc.scalar.activation(
                            q2s[:, blk], q2[:, blk],
                            mybir.ActivationFunctionType.Copy,
                            scale=rq[:, hh:hh + 1])
                    # token->pack transposes on the DMA XBAR (dest contiguous)
                    # q on the SP queue, k on the ACT queue: halves the
                    # serialized transpose chain that gated PH1's tail.
                    nc.sync.dma_start_transpose(out=qpack[:, nt], in_=q2s)
                    nc.sync.dma_start_transpose(out=kpack[:, nt], in_=k2s)

            # Transition warm-keeper: the PH1->PH2 PSUM handoff idles the PE
            # ~4-5us, long enough for a HAM MID window to re-throttle to
            # 1.2 GHz.  A dozen dummy matmuls sit in the PE's in-order queue
            # between the phases and keep the activity window alive; they
            # start as soon as the first PH1 bank drains.
            with tc.tile_pool(name="warm2", bufs=1, space="PSUM") as pw2:
                w2ps = pw2.tile([128, 512], F32)
                for _ in range(12):
                    nc.tensor.matmul(w2ps, wz[:, 0:128], wz,
                                     start=True, stop=True)

            # ---------------- PH2: attention ----------------
            # st 3-deep + pv single-buffered: lets the DVE es-steps overlap
            # ACT exp-steps; PV emission lags one kt so a late es never
            # blocks the next S^T in the in-order PE queue.
            with (
                tc.tile_pool(name="stp", bufs=3, space="PSUM") as pst,
                tc.tile_pool(name="pvp", bufs=1, space="PSUM") as ppv,
                tc.tile_pool(name="esp", bufs=4) as pes,
                tc.tile_pool(name="zcp", bufs=2) as pzc,
                tc.tile_pool(name="fsp", bufs=2) as pfs,
            ):
                for hh in range(HPC):
                    for qc in range(QC):
                        t0 = qc * 8
                        pv_ps = ppv.tile([128, 1024], F32, tag="pv")
                        zacc = pzc.tile([128, 2, 1024], MMD, tag="zc")
                        es_prev = None

                        def emit_pv(kt, es_t):
                            for hf in range(2):
                                nc.tensor.matmul(
                                    pv_ps[:, hf * 512:(hf + 1) * 512],
                                    vpack[:, kt, hh * 128:(hh + 1) * 128],
                                    es_t[:, hf * 512:(hf + 1) * 512],
                                    start=(kt == 0), stop=(kt == NT - 1))

                        for kt in range(NT):
                            st_ps = pst.tile([128, 1024], F32, tag="st")
                            for hf in range(2):
                                nc.tensor.matmul(
                                    st_ps[:, hf * 512:(hf + 1) * 512],
                                    kpack[:, kt, hh],
                                    qpack[:, t0 + hf * 4:t0 + hf * 4 + 4, hh],
                                    start=True, stop=True)
                            es = pes.tile([128, 1024], MMD, tag="es")
                            if kt in SCH_KT:
                                nc.vector.tensor_scalar(
                                    es.bitcast(mybir.dt.int16), st_ps,
                                    rkA_sb[:, kt, hh:hh + 1], SCH_B,
                                    op0=mybir.AluOpType.mult,
                                    op1=mybir.AluOpType.add)
                            else:
                                nc.scalar.activation(
                                    es, st_ps,
                                    mybir.ActivationFunctionType.Exp,
                                    scale=rk_sb[:, kt, hh:hh + 1])
                            if es_prev is not None:
                                emit_pv(kt - 1, es_prev)
                            es_prev = es
                            if kt < 2:
                                nc.vector.tensor_copy(zacc[:, kt], es)
                            else:
                                nc.vector.tensor_add(zacc[:, kt % 2],
                                                     zacc[:, kt % 2], es)
                        emit_pv(NT - 1, es_prev)
                        final_sb = pfs.tile([128, 1024], F32, tag="fin")
                        for hf in range(2):
                            sl = slice(hf * 512, (hf + 1) * 512)
                            nc.vector.tensor_copy(final_sb[:, sl], pv_ps[:, sl])
                            nc.sync.dma_start(out=outT_d[hh, qc, :, sl],
                                              in_=final_sb[:, sl])
                        for par in range(2):
                            nc.sync.dma_start(out=zac_d[hh, qc, :, par],
                                              in_=zacc[:, par])
    nc.compile()
    return nc


def _host_prep(x_real, x_imag, w_qkv, b_qkv, qn_w, kn_w):
    """Build the 8 per-core input maps (numpy only)."""
    qw_col = np.tile(qn_w, H)[:, None]            # [E,1] complex
    kw_col = np.tile(kn_w, H)[:, None]
    wq = w_qkv[0 * E:1 * E] * qw_col
    wk = w_qkv[1 * E:2 * E] * kw_col
    wv = w_qkv[2 * E:3 * E]
    bq = b_qkv[0 * E:1 * E] * qw_col[:, 0]
    bk = b_qkv[1 * E:2 * E] * kw_col[:, 0]

    import ml_dtypes
    bf16 = ml_dtypes.bfloat16
    in_maps = []
    for c in range(8):
        b = c // 2
        h0 = HPC * (c % 2)
        # weight tiles: w[pack, plane, e, col] with col = hh*128 + 2d (+1)
        w_arr = np.zeros((3, 2, E, 384), dtype=np.float32)
        b_arr = np.zeros((1, 2, 384), dtype=np.float32)
        for p, wm in enumerate((wq, wk, wv)):
            for hh in range(HPC):
                rows = slice((h0 + hh) * D, (h0 + hh + 1) * D)
                wr = wm[rows].real.T.astype(np.float32)   # [E, D]
                wi = wm[rows].imag.T.astype(np.float32)
                cs = slice(hh * 128, hh * 128 + 128)
                w_arr[p, 0, :, cs.start:cs.stop:2] = wr
                w_arr[p, 0, :, cs.start + 1:cs.stop:2] = wi
                w_arr[p, 1, :, cs.start:cs.stop:2] = -wi
                w_arr[p, 1, :, cs.start + 1:cs.stop:2] = wr
        for p, bm in enumerate((bq, bk)):
            for hh in range(HPC):
                rows = slice((h0 + hh) * D, (h0 + hh + 1) * D)
                br = bm[rows].real.astype(np.float32)
                bi = bm[rows].imag.astype(np.float32)
                cs = slice(hh * 128, hh * 128 + 128)
                b_arr[0, p, cs.start:cs.stop:2] = br
                b_arr[0, p, cs.start + 1:cs.stop:2] = bi
        in_maps.append({
            "xt_r": np.ascontiguousarray(x_real[b].T).astype(bf16),
            "xt_i": np.ascontiguousarray(x_imag[b].T).astype(bf16),
            "w": w_arr.astype(bf16),
            "bias": b_arr,
        })
    return in_maps


def _run(x_real, x_imag, w_qkv, b_qkv, qn_w, kn_w, trace=False):
    import time as _t
    if "nc" not in _prog_cache:
        t0 = _t.time()
        _prog_cache["nc"] = build_program()
        print(f"[kernel] program built in {_t.time() - t0:.1f}s", flush=True)
    nc = _prog_cache["nc"]
    t0 = _t.time()
    in_maps = _host_prep(x_real, x_imag, w_qkv, b_qkv, qn_w, kn_w)
    print(f"[kernel] host prep {_t.time() - t0:.1f}s", flush=True)
    t0 = _t.time()
    try:
        res = run_bass_kernel_spmd(nc, in_maps, list(range(8)), trace=trace)
    except Exception as e:
        if not trace:
            raise
        print(f"[kernel] trace run failed ({e!r}); retrying without trace",
              flush=True)
        res = run_bass_kernel_spmd(nc, in_maps, list(range(8)), trace=False)
    print(f"[kernel] device run {_t.time() - t0:.1f}s", flush=True)

    full = np.zeros((B, N, E), dtype=np.complex64)
    bv = b_qkv[2 * E:3 * E]                       # v bias, applied on host
    for c in range(8):
        b = c // 2
        h0 = HPC * (c % 2)
        outT = res.results[c]["outT"].astype(np.float32)  # [3,2,128,1024]
        zac = res.results[c]["zac"].astype(np.float32)    # [3,2,128,2,1024]
        for hh in range(HPC):
            for qc in range(QC):
                z = zac[hh, qc].sum(axis=(0, 1))          # [1024]
                o = outT[hh, qc] / z[None, :]             # [128, 1024]
                oc = (o[0::2] + 1j * o[1::2]).T           # [1024, 64]
                h = h0 + hh
                full[b, qc * 1024:(qc + 1) * 1024,
                     h * D:(h + 1) * D] = oc + bv[h * D:(h + 1) * D]
    return full, res


def kernel(x_real, x_imag, w_qkv, b_qkv, qn_w, kn_w):
    full, _ = _run(x_real, x_imag, w_qkv, b_qkv, qn_w, kn_w, trace=False)
    return full


def kernel_profiled(x_real, x_imag, w_qkv, b_qkv, qn_w, kn_w):
    return _run(x_real, x_imag, w_qkv, b_qkv, qn_w, kn_w, trace=True)



# revision 4
# speedup vs baseline: 1.0151x; 1.0151x over previous
"""Complex-valued attention kernel for Trainium2, SPMD over 8 NeuronCores.

Problem (hardcoded shapes): B=4, N=2048, E=384, H=6, D=64, complex64.
  qkv = x @ w_qkv^T + b_qkv; q,k complex-RMSNormed; scores =
  Re(q conj(k)^T)/8; attn = softmax(scores); out = attn @ v.
Sharding: core c handles batch c//2, heads 3*(c%2)..+2 (24 over 8, 3 each).

Design (HW ~183us, from a 196us two-phase baseline; all trace-driven):
  - PH1: QKV proj token-major, bias in the PSUM->SBUF evac adds (DVE);
    q scaled by rq on ACT copies; k UNSCALED (rk folds into the exp's
    per-partition scale).  Pack transposes on the DMA XBAR; odd-tile q
    transposes via the ACT HWDGE queue.  V evac on DVE.  V bias on host.
  - PH2 per (head, 1024-q chunk) x 16 kv tiles: S^T = kpack^T qpack,
    es = exp(rk*S^T), PV accumulation.
  - Schraudolph exp on DVE for kv tiles {1,4,6,9,11,14}: i16 =
    round(S^T*(rk*A)+B) bitcast int16->bf16 (A=2^7/ln2, B=127*2^7-7.4);
    HW-verified ~2% exp accuracy, total rel err 6.9e-3 vs 2e-2 gate.
  - NO on-device Z: every es tile is DMA'd raw to DRAM (the DMA/Sync
    queues are ~85% idle in PH2) and the host sums the softmax
    denominator.  This deleted the 51us DVE zacc chain, letting DVE
    absorb 36 Schraudolph tiles; PH2 is then PE-paced (~87us busy, MM
    start-to-start 215ns = the bf16 stream floor with LDW overlapped).
  - PV emission lags one kv step and each block's last PV + pv evac is
    deferred past the next block's first S^T, so a late DVE-es never
    blocks the in-order PE queue (exp-chain gaps 22us -> <5us).
  - PSUM: ph1 8x1 banks -> st 3x2 + pv 1x2 (lazy-alloc across the
    deferred flush) = 8 banks.
  - Keep the 14-MM initial warm and 12-MM transition warm blocks exactly
    as-is: removing the transition one costs +2.6us (HAM K=4/8), and
    raising the initial count to 22 BROKE CORRECTNESS (rel err 14.6).
Measured failures (do not reintroduce): fp8 anywhere (sim: proj 4.1e-2,
es+v 2.1e-2), weaving sqrt-bearing PH1 tiles into PH2 (43us ACT-table
thrash; sqrt and exp share no table set), gpsimd SWDGE bulk loads
(~2.3us/dispatch), x/w loads on the ACT queue (+6us), QK-first-then-V
PH1 split (+8.6us, DVE-evac paced), tensor_tensor_reduce (HW fault),
gpsimd ops reading PSUM (crash), rearrange-view reduce (NaN on HW).
"""

import numpy as np

import concourse.bass as bass
import concourse.tile as tile
from concourse import bacc, mybir
from concourse.bass_utils import run_bass_kernel_spmd

B, N, E, H, D = 4, 2048, 384, 6, 64
EPS = 1e-6
HPC = 3            # heads per core
NT = N // 128      # 16 token tiles
KT = E // 128      # 3 contraction tiles
QC = 2             # q chunks of 1024
F32 = mybir.dt.float32
MMD = mybir.dt.bfloat16

_prog_cache = {}

# Schraudolph exp on DVE for these kv tiles of every (head, q-chunk):
# i16 = round(S^T * (rk*A) + SCH_B), bitcast int16->bf16 gives exp to ~2%.
# Offloads ACT (the PH2 pacer) onto DVE slack.
SCH_A = float(2.0 ** 7 / np.log(2.0))
SCH_B = float(127 * 2 ** 7 - 7.4)
SCH_KT = (1, 4, 6, 9, 11, 14)


def _widx(p, a, k):
    return (p * 2 + a) * KT + k


def build_program():
    nc = bacc.Bacc(
        "TRN2", target_bir_lowering=False, debug=False, num_devices=8)
    xt_r = nc.declare_dram_parameter("xt_r", [E, N], MMD, isOutput=False)
    xt_i = nc.declare_dram_parameter("xt_i", [E, N], MMD, isOutput=False)
    w_in = nc.declare_dram_parameter("w", [3, 2, E, 384], MMD, isOutput=False)
    b_in = nc.declare_dram_parameter("bias", [1, 2, 384], F32, isOutput=False)
    outT_d = nc.declare_dram_parameter("outT", [HPC, QC, 128, 1024], F32,
                                       isOutput=True)
    # raw exp tiles stream to DRAM; the softmax denominator Z is summed on
    # the host (kills the 51.5us zacc chain on the DVE, whose freed slack
    # takes 6 Schraudolph tiles per chunk instead of 4)
    es_d = nc.declare_dram_parameter("es", [HPC, QC, NT, 128, 1024], MMD,
                                     isOutput=True)

    with tile.TileContext(nc) as tc:
        with tc.tile_pool(name="persist", bufs=1) as pp:
            bias_row = pp.tile([1, 2, 384], F32)
            nc.sync.dma_start(out=bias_row, in_=b_in[:])
            bias_sb = pp.tile([128, 2, 384], F32)
            nc.gpsimd.partition_broadcast(bias_sb, bias_row)
            w_sb = pp.tile([128, 3 * 2 * KT, 384], MMD)

            # packs: [d2, token-tile, head, token-in-tile]
            qpack = pp.tile([128, NT, HPC, 128], MMD)
            kpack = pp.tile([128, NT, HPC, 128], MMD)
            vpack = pp.tile([128, NT, 384], MMD)   # [token, kv-tile, head*128]
            rk_sb = pp.tile([128, NT, HPC], F32)   # per-kv-token k norm scale
            rkA_sb = pp.tile([128, NT, HPC], F32)  # rk * SCH_A (DVE exp tiles)
            eps_q = pp.tile([128, 1], F32)
            eps_k = pp.tile([128, 1], F32)
            nc.vector.memset(eps_q, 64.0 * EPS)
            nc.vector.memset(eps_k, EPS)

            xt_sb = pp.tile([128, 2, KT, N], MMD)
            NXC = 8  # x dma chunks

            def _x_chunk(xc, split):
                # split=True: x_r via the ACT HWDGE queue, x_i via SP, so
                # the SP queue has room for the 32 pack transposes.
                sl = slice(xc * (N // NXC), (xc + 1) * (N // NXC))
                eng_r = nc.scalar if split else nc.sync
                eng_r.dma_start(
                    out=xt_sb[:, 0, :, sl],
                    in_=xt_r[:, sl].rearrange("(k q) n -> q k n", q=128),
                )
                nc.sync.dma_start(
                    out=xt_sb[:, 1, :, sl],
                    in_=xt_i[:, sl].rearrange("(k q) n -> q k n", q=128),
                )

            # x chunk 0 first (gates tile 0), then weights in 6 chunks
            # ordered by per-tile MM consumption ((k,a) inner loops) so the
            # first matmuls start ~4us in instead of waiting for all 3.5MB,
            # then the remaining x chunks.
            _x_chunk(0, split=False)
            for k in range(KT):
                for a in range(2):
                    nc.sync.dma_start(
                        out=w_sb[:, a * KT + k::2 * KT],
                        in_=w_in[:, a, k * 128:(k + 1) * 128, :].rearrange(
                            "p q c -> q p c"),
                    )
            for xc in range(1, NXC):
                _x_chunk(xc, split=True)

            # HAM pre-warm: dummy matmuls keep the PE busy through the
            # initial DMA wait so the 4096-cycle activity window fires and
            # PH1's real matmuls run at K=8/8 (2.4 GHz) from the start.
            wz = pp.tile([128, 512], MMD)
            nc.vector.memset(wz, 0.0)
            with tc.tile_pool(name="warm", bufs=1, space="PSUM") as pwm:
                warm_ps = pwm.tile([128, 512], F32)
                for _ in range(14):
                    nc.tensor.matmul(warm_ps, wz[:, 0:128], wz,
                                     start=True, stop=True)

            # ---------------- PH1: QKV + RMS norm + packing ----------------
            with (
                tc.tile_pool(name="ph1ps", bufs=8, space="PSUM") as pps,
                tc.tile_pool(name="ph1t", bufs=4) as pt1,
            ):
                for nt in range(NT):
                    psq = pps.tile([128, 384], F32, tag="ps")
                    psk = pps.tile([128, 384], F32, tag="ps")
                    psv = pps.tile([128, 384], F32, tag="ps")
                    for k in range(KT):
                        for a in range(2):
                            lhs = xt_sb[:, a, k, nt * 128:(nt + 1) * 128]
                            st = (k == 0 and a == 0)
                            sp = (k == KT - 1 and a == 1)
                            nc.tensor.matmul(psq, lhs, w_sb[:, _widx(0, a, k)],
                                             start=st, stop=sp)
                            nc.tensor.matmul(psk, lhs, w_sb[:, _widx(1, a, k)],
                                             start=st, stop=sp)
                            nc.tensor.matmul(psv, lhs, w_sb[:, _widx(2, a, k)],
                                             start=st, stop=sp)
                    # V: plain PSUM->SBUF bf16 evacuation (bias on host);
                    # on DVE to keep the ACT queue under the PE pace in PH1.
                    nc.vector.tensor_copy(vpack[:, nt], psv)
                    # Q/K: add bias during evacuation
                    q2 = pt1.tile([128, 384], F32, tag="q2")
                    k2s = pt1.tile([128, 384], MMD, tag="k2s")
                    nc.vector.tensor_add(q2, psq, bias_sb[:, 0])
                    nc.vector.tensor_add(k2s, psk, bias_sb[:, 1])
                    # sum of squares per head: squares on gpsimd, reduce DVE
                    scr = pt1.tile([128, 384], MMD, tag="scr")
                    scrk = pt1.tile([128, 384], MMD, tag="scrk")
                    msq = pt1.tile([128, HPC], F32, tag="msq")
                    msk = pt1.tile([128, HPC], F32, tag="msk")
                    nc.scalar.square(scr, q2)
                    nc.vector.tensor_mul(scrk, k2s, k2s)
                    for hh in range(HPC):
                        blk = slice(hh * 128, (hh + 1) * 128)
                        nc.vector.reduce_sum(msq[:, hh:hh + 1], scr[:, blk],
                                             axis=mybir.AxisListType.X)
                        nc.vector.reduce_sum(msk[:, hh:hh + 1], scrk[:, blk],
                                             axis=mybir.AxisListType.X)
                    # q: rq = 1/(8 sqrt(ms+eps)) = 1/sqrt(sum_sq + 64 eps)
                    # k: rk = 1/sqrt(ms+eps)     = 1/sqrt((sum_sq)/64 + eps)
                    s8q = pt1.tile([128, HPC], F32, tag="s8q")
                    s8k = pt1.tile([128, HPC], F32, tag="s8k")
                    nc.scalar.activation(s8q, msq,
                                         mybir.ActivationFunctionType.Sqrt,
                                         bias=eps_q, scale=1.0)
                    nc.scalar.activation(s8k, msk,
                                         mybir.ActivationFunctionType.Sqrt,
                                         bias=eps_k, scale=1.0 / 64.0)
                    rq = pt1.tile([128, HPC], F32, tag="rq")
                    nc.vector.reciprocal(rq, s8q)
                    nc.vector.reciprocal(rk_sb[:, nt], s8k)
                    nc.vector.tensor_scalar_mul(rkA_sb[:, nt], rk_sb[:, nt],
                                                SCH_A)
                    # scale q by rq (per-head per-token) while casting to bf16
                    q2s = pt1.tile([128, 384], MMD, tag="q2s")
                    for hh in range(HPC):
                        blk = slice(hh * 128, (hh + 1) * 128)
                        nc.scalar.activation(
                            q2s[:, blk], q2[:, blk],
                            mybir.ActivationFunctionType.Copy,
                            scale=rq[:, hh:hh + 1])
                    # token->pack transposes on the DMA XBAR (dest contiguous)
                    # q on the SP queue, k on the ACT queue: halves the
                    # serialized transpose chain that gated PH1's tail.
                    # odd-tile q transposes ride the ACT HWDGE queue: the SP
                    # queue (x_i + w loads + 24 transposes) was PH1's tail.
                    qeng = nc.scalar if (nt % 2) else nc.sync
                    qeng.dma_start_transpose(out=qpack[:, nt], in_=q2s)
                    nc.sync.dma_start_transpose(out=kpack[:, nt], in_=k2s)

            # Transition warm-keeper: the PH1->PH2 PSUM handoff idles the PE
            # ~4-5us, long enough for a HAM MID window to re-throttle to
            # 1.2 GHz.  A dozen dummy matmuls sit in the PE's in-order queue
            # between the phases and keep the activity window alive; they
            # start as soon as the first PH1 bank drains.
            with tc.tile_pool(name="warm2", bufs=1, space="PSUM") as pw2:
                w2ps = pw2.tile([128, 512], F32)
                for _ in range(12):
                    nc.tensor.matmul(w2ps, wz[:, 0:128], wz,
                                     start=True, stop=True)

            # ---------------- PH2: attention ----------------
            # st 3-deep + pv single-buffered: lets the DVE es-steps overlap
            # ACT exp-steps; PV emission lags one kt so a late es never
            # blocks the next S^T in the in-order PE queue.
            with (
                tc.tile_pool(name="stp", bufs=3, space="PSUM") as pst,
                tc.tile_pool(name="pvp", bufs=1, space="PSUM") as ppv,
                tc.tile_pool(name="esp", bufs=6) as pes,
                tc.tile_pool(name="fsp", bufs=2) as pfs,
            ):
                # flush_prev: the previous block's last PV + pv evacuation,
                # emitted AFTER the next block's first S^T so the exp chain
                # never stalls behind it at block boundaries.
                flush_prev = [None]

                for hh in range(HPC):
                    for qc in range(QC):
                        t0 = qc * 8
                        # pv tile allocated lazily at kt==1 (after the
                        # deferred flush of the previous block releases the
                        # single pv buffer)
                        pv_ps = None
                        es_prev = None

                        def emit_pv(kt, es_t, pv_t, h):
                            for hf in range(2):
                                nc.tensor.matmul(
                                    pv_t[:, hf * 512:(hf + 1) * 512],
                                    vpack[:, kt, h * 128:(h + 1) * 128],
                                    es_t[:, hf * 512:(hf + 1) * 512],
                                    start=(kt == 0), stop=(kt == NT - 1))

                        for kt in range(NT):
                            st_ps = pst.tile([128, 1024], F32, tag="st")
                            for hf in range(2):
                                nc.tensor.matmul(
                                    st_ps[:, hf * 512:(hf + 1) * 512],
                                    kpack[:, kt, hh],
                                    qpack[:, t0 + hf * 4:t0 + hf * 4 + 4, hh],
                                    start=True, stop=True)
                            es = pes.tile([128, 1024], MMD, tag="es")
                            if kt in SCH_KT:
                                nc.vector.tensor_scalar(
                                    es.bitcast(mybir.dt.int16), st_ps,
                                    rkA_sb[:, kt, hh:hh + 1], SCH_B,
                                    op0=mybir.AluOpType.mult,
                                    op1=mybir.AluOpType.add)
                            else:
                                nc.scalar.activation(
                                    es, st_ps,
                                    mybir.ActivationFunctionType.Exp,
                                    scale=rk_sb[:, kt, hh:hh + 1])
                            if kt == 0 and flush_prev[0] is not None:
                                flush_prev[0]()
                                flush_prev[0] = None
                            if es_prev is not None:
                                if pv_ps is None:
                                    pv_ps = ppv.tile([128, 1024], F32,
                                                     tag="pv")
                                emit_pv(kt - 1, es_prev, pv_ps, hh)
                            es_prev = es
                            nc.sync.dma_start(out=es_d[hh, qc, kt], in_=es)

                        def make_flush(kt_l, es_l, pv_l, h_l, qc_l):
                            def _flush():
                                emit_pv(kt_l, es_l, pv_l, h_l)
                                final_sb = pfs.tile([128, 1024], F32,
                                                    tag="fin")
                                for hf in range(2):
                                    sl = slice(hf * 512, (hf + 1) * 512)
                                    nc.vector.tensor_copy(final_sb[:, sl],
                                                          pv_l[:, sl])
                                    nc.sync.dma_start(
                                        out=outT_d[h_l, qc_l, :, sl],
                                        in_=final_sb[:, sl])
                            return _flush

                        flush_prev[0] = make_flush(NT - 1, es_prev, pv_ps,
                                                   hh, qc)
                flush_prev[0]()
    nc.compile()
    return nc


def _host_prep(x_real, x_imag, w_qkv, b_qkv, qn_w, kn_w):
    """Build the 8 per-core input maps (numpy only)."""
    qw_col = np.tile(qn_w, H)[:, None]            # [E,1] complex
    kw_col = np.tile(kn_w, H)[:, None]
    wq = w_qkv[0 * E:1 * E] * qw_col
    wk = w_qkv[1 * E:2 * E] * kw_col
    wv = w_qkv[2 * E:3 * E]
    bq = b_qkv[0 * E:1 * E] * qw_col[:, 0]
    bk = b_qkv[1 * E:2 * E] * kw_col[:, 0]

    import ml_dtypes
    bf16 = ml_dtypes.bfloat16
    in_maps = []
    for c in range(8):
        b = c // 2
        h0 = HPC * (c % 2)
        # weight tiles: w[pack, plane, e, col] with col = hh*128 + 2d (+1)
        w_arr = np.zeros((3, 2, E, 384), dtype=np.float32)
        b_arr = np.zeros((1, 2, 384), dtype=np.float32)
        for p, wm in enumerate((wq, wk, wv)):
            for hh in range(HPC):
                rows = slice((h0 + hh) * D, (h0 + hh + 1) * D)
                wr = wm[rows].real.T.astype(np.float32)   # [E, D]
                wi = wm[rows].imag.T.astype(np.float32)
                cs = slice(hh * 128, hh * 128 + 128)
                w_arr[p, 0, :, cs.start:cs.stop:2] = wr
                w_arr[p, 0, :, cs.start + 1:cs.stop:2] = wi
                w_arr[p, 1, :, cs.start:cs.stop:2] = -wi
                w_arr[p, 1, :, cs.start + 1:cs.stop:2] = wr
        for p, bm in enumerate((bq, bk)):
            for hh in range(HPC):
                rows = slice((h0 + hh) * D, (h0 + hh + 1) * D)
                br = bm[rows].real.astype(np.float32)
                bi = bm[rows].imag.astype(np.float32)
                cs = slice(hh * 128, hh * 128 + 128)
                b_arr[0, p, cs.start:cs.stop:2] = br
                b_arr[0, p, cs.start + 1:cs.stop:2] = bi
        in_maps.append({
            "xt_r": np.ascontiguousarray(x_real[b].T).astype(bf16),
            "xt_i": np.ascontiguousarray(x_imag[b].T).astype(bf16),
            "w": w_arr.astype(bf16),
            "bias": b_arr,
        })
    return in_maps


def _run(x_real, x_imag, w_qkv, b_qkv, qn_w, kn_w, trace=False):
    import time as _t
    if "nc" not in _prog_cache:
        t0 = _t.time()
        _prog_cache["nc"] = build_program()
        print(f"[kernel] program built in {_t.time() - t0:.1f}s", flush=True)
    nc = _prog_cache["nc"]
    t0 = _t.time()
    in_maps = _host_prep(x_real, x_imag, w_qkv, b_qkv, qn_w, kn_w)
    print(f"[kernel] host prep {_t.time() - t0:.1f}s", flush=True)
    t0 = _t.time()
    try:
        res = run_bass_kernel_spmd(nc, in_maps, list(range(8)), trace=trace)
    except Exception as e:
        if not trace:
            raise
        print(f"[kernel] trace run failed ({e!r}); retrying without trace",
              flush=True)
        res = run_bass_kernel_spmd(nc, in_maps, list(range(8)), trace=False)
    print(f"[kernel] device run {_t.time() - t0:.1f}s", flush=True)

    full = np.zeros((B, N, E), dtype=np.complex64)
    bv = b_qkv[2 * E:3 * E]                       # v bias, applied on host
    for c in range(8):
        b = c // 2
        h0 = HPC * (c % 2)
        outT = res.results[c]["outT"].astype(np.float32)  # [3,2,128,1024]
        es_arr = res.results[c]["es"]                     # [3,2,16,128,1024]
        for hh in range(HPC):
            for qc in range(QC):
                z = es_arr[hh, qc].astype(np.float32).sum(axis=(0, 1))
                o = outT[hh, qc] / z[None, :]             # [128, 1024]
                oc = (o[0::2] + 1j * o[1::2]).T           # [1024, 64]
                h = h0 + hh
                full[b, qc * 1024:(qc + 1) * 1024,
                     h * D:(h + 1) * D] = oc + bv[h * D:(h + 1) * D]
    return full, res


def kernel(x_real, x_imag, w_qkv, b_qkv, qn_w, kn_w):
    full, _ = _run(x_real, x_imag, w_qkv, b_qkv, qn_w, kn_w, trace=False)
    return full


def kernel_profiled(x_real, x_imag, w_qkv, b_qkv, qn_w, kn_w):
    return _run(x_real, x_imag, w_qkv, b_qkv, qn_w, kn_w, trace=True)

